# revision 5
# baseline (speedup 1.0000x reference)
"""NVFP4-style activation quantizer on 8 TRN2 NeuronCores (raw bass).

Single-pass over HBM: x streams in once; an fp16 copy (x16) of the whole
shard stays resident in SBUF, so the post-AllReduce quantize pass re-reads
nothing from HBM.  Output is written once.  ~67MB HBM traffic per core
instead of the 100MB of a two-pass design.

Reference semantics (per 16-element block, fp32):
    s_t  = max|x| / (6*448)                      (global, needs all-reduce)
    m_b  = max|x| over block
    s_b  = fp8_e4m3_roundtrip(6 * s_t / m_b)
    out  = sign(x) * fp4_121(|x|/s_t * s_b) / s_b * s_t

Device algorithm per element (signed, select-free), fp16 magic rounding:
    y16 = fp16(x16 * c)        c = s_b / s_t  (per block, fp32)
    p   = (bits(y16) & 0x7C00) + 0x2600        (u16 ops)
    B   = max(p, 0x6200)                       (= 768*max(2^e,1) as fp16)
    t   = fp16(y16 + B)        (fp16 RNE add rounds y to the fp4 grid)
    nq  = B - t                (exact; = -fp4_121(y)*sign)
    out = nq * nic             nic = -s_t/s_b  (per block, fp16)

The fp16 magic add reproduces fp4_121 rounding because the 1-2-1 grid
step is 2^(e-1) clamped to >= 0.5 == ulp_fp16(768*max(2^e,1)).  The u16
tensor_scalar ops hit the DVE 4x perf mode and the fp16 adds the 2x mode.
Storing x as fp16 + the fp16 multiply flips ~0.1% of elements by one
grid step; measured rel_err vs the fp32 reference on seed-0 data: 1.16e-2.

Engine split:
  ACT    input DMAs + fp16 conversion of x into resident x16
  DVE    pass-A block maxes, per-block scales (eighth-chunks, interleaved
         with quantize tiles), and the 5-op fp16 quantize chain
  POOL   AllReduce (warmed up) + ALL final per-block multiplies
  SYNC   collective staging + output DMAs
"""

import numpy as np

FULL_SHAPE = (4, 4096, 4096)
N_CORES = 8
P = 128
TOTAL = 4 * 4096 * 4096
L = TOTAL // (N_CORES * P)   # 65536 elements per partition per core
NBLK = L // 16               # 4096 blocks per partition

M16 = 0x7C00                 # fp16 exponent mask
A16 = 0x2600                 # +0x2600: exponent +9, set mantissa bit -> *768
MN16 = 0x6200                # bits of fp16 768.0


def build_nc(n_cores=N_CORES, FA=2048, FB=1024, n_xa=2, n_o=2,
             n_chunks=8, dve6_mod=0):
    """dve6_mod=0: all final multiplies on Pool; k>0: DVE keeps every k-th."""
    from contextlib import ExitStack

    import concourse.bass as bass
    from concourse import mybir

    f32 = mybir.dt.float32
    f16 = mybir.dt.float16
    u16 = mybir.dt.uint16
    f8 = mybir.dt.float8e4

    TA = L // FA                 # pass-A tiles (32)
    TB = L // FB                 # pass-B tiles (64)
    fbB = FB // 16               # blocks per pass-B tile (64)
    QN = NBLK // n_chunks        # blocks per scale chunk (512)
    TPQ = TB // n_chunks         # pass-B tiles per chunk (8)
    assert L % FA == 0 and L % FB == 0 and NBLK % n_chunks == 0
    assert TB % n_chunks == 0 and TPQ % 2 == 0

    def e_dve(t):                # final multiply on DVE for tile t?
        return dve6_mod > 0 and t % dve6_mod == dve6_mod - 1

    # pool's cumulative tag schedule (DVE traced first, needs these)
    ptag6 = [0] * TB
    pc = 1                       # warm-up memset increments s_pool once
    for t in range(TB):
        if not e_dve(t):
            pc += 1
        ptag6[t] = pc

    nc = bass.Bass(num_devices=n_cores, debug=False)
    x_ext = nc.declare_dram_parameter("x", [P, L], f32, isOutput=False)
    out_ext = nc.declare_dram_parameter("out", [P, L], f32, isOutput=True)
    cc_in = nc.dram_tensor("cc_in", [1, 128], f32)
    cc_out = nc.dram_tensor("cc_out", [1, 128], f32, addr_space="Shared")
    cc_warm_in = nc.dram_tensor("cc_warm_in", [1, 128], f32)
    cc_warm_out = nc.dram_tensor("cc_warm_out", [1, 128], f32,
                                 addr_space="Shared")

    with ExitStack() as ctx:
        def sem(name):
            return ctx.enter_context(nc.semaphore(name))

        def sbuf(name, shape, dt=f32):
            return ctx.enter_context(nc.sbuf_tensor(name, shape, dt))

        s_xa = [sem(f"s_xa{i}") for i in range(n_xa)]   # input DMAs   (+16)
        s_ob = [sem(f"s_ob{i}") for i in range(n_o)]    # output DMAs  (+16)
        s_cdma = sem("s_cdma")   # collective staging DMAs             (+16)
        s_act = sem("s_act")     # ACT x16 conversions                 (+1)
        s_dve = sem("s_dve")     # tagged DVE ops                      (+1)
        s_pool = sem("s_pool")   # pool ops                            (+1)
        s_cc = sem("s_cc")       # collectives                         (+1)
        s_warm = sem("s_warm")   # warm-up staging dma                 (+16)

        xa = [sbuf(f"xa{i}", [P, FA]) for i in range(n_xa)]
        x16 = sbuf("x16", [P, L], f16)        # resident fp16 shard (128KB)
        m_t = sbuf("m_t", [P, NBLK])          # block max m -> c
        r_t = sbuf("r_t", [P, NBLK])          # rm=1/m (AR window) -> inv
        nic16 = sbuf("nic16", [P, NBLK], f16)
        f8_q = sbuf("f8_q", [P, QN], f8)      # chunk temps
        sb_q = sbuf("sb_q", [P, QN])
        t1_q = sbuf("t1_q", [P, QN])
        yb = [sbuf(f"yb{i}", [P, FB], f16) for i in range(2)]
        pb = [sbuf(f"pb{i}", [P, FB], u16) for i in range(2)]
        ob = [sbuf(f"ob{i}", [P, FB]) for i in range(n_o)]
        gall = sbuf("gall", [P, 128])
        mx_t = sbuf("mx_t", [P, 1])
        g128 = sbuf("g128", [P, 1])
        st_t = sbuf("st_t", [P, 1])
        rt_t = sbuf("rt_t", [P, 1])
        nst_t = sbuf("nst_t", [P, 1])

        tagA = [0] * TA
        tag_nq = [0] * TB        # last chain op per tile (nq in yb slot)
        tag6 = [0] * TB          # DVE final-multiply tags
        K_mx = [0]

        def b3(ap):
            return ap.rearrange("p (b s) -> p b s", s=16)

        def bc(ap_slice, nb):
            return ap_slice.unsqueeze(-1).broadcast_to([P, nb, 16])

        with nc.Block() as block:

            @block.vector
            def _(dve):
                cnt = 0

                def tag(ins):
                    nonlocal cnt
                    ins.then_inc(s_dve)
                    cnt += 1
                    return cnt

                # ---- pass A: per-block abs max ----
                fbA = FA // 16
                for t in range(TA):
                    dve.wait_ge(s_xa[t % n_xa], 16 * (t // n_xa + 1))
                    tagA[t] = tag(dve.tensor_reduce(
                        out=m_t[:, t * fbA:(t + 1) * fbA],
                        in_=b3(xa[t % n_xa][:]),
                        axis=mybir.AxisListType.X,
                        op=mybir.AluOpType.max,
                        apply_absolute_value=True,
                    ))
                dve.wait_ge(s_dve, tagA[TA - 1])
                K_mx[0] = tag(dve.tensor_reduce(
                    out=mx_t[:], in_=m_t[:], axis=mybir.AxisListType.X,
                    op=mybir.AluOpType.max))
                dve.wait_ge(s_dve, K_mx[0])
                # rm = 1/m for the whole shard, hidden in the AllReduce window
                h = NBLK // 2
                k_rm0 = tag(dve.reciprocal(r_t[:, 0:h], m_t[:, 0:h]))
                k_rm1 = tag(dve.reciprocal(r_t[:, h:NBLK], m_t[:, h:NBLK]))

                # ---- post-AllReduce scalars ----
                dve.wait_ge(s_cdma, 32)        # gall loaded (bcast DMA)
                k1 = tag(dve.tensor_reduce(
                    out=g128[:], in_=gall[:], axis=mybir.AxisListType.X,
                    op=mybir.AluOpType.max))
                dve.wait_ge(s_dve, k1)
                k2 = tag(dve.tensor_scalar(
                    st_t[:], g128[:], 1.0 / 2688.0, None,
                    op0=mybir.AluOpType.mult))
                dve.wait_ge(s_dve, k2)
                k3 = tag(dve.reciprocal(rt_t[:], st_t[:]))
                k4 = tag(dve.tensor_scalar(
                    nst_t[:], st_t[:], -1.0, None, op0=mybir.AluOpType.mult))
                dve.wait_ge(s_dve, k4)
                dve.wait_ge(s_dve, k3)
                dve.wait_ge(s_dve, k_rm0)
                dve.wait_ge(s_dve, k_rm1)

                # ---- per-chunk scales + pass-B tiles ----
                for q in range(n_chunks):
                    sl = slice(q * QN, (q + 1) * QN)
                    a = tag(dve.tensor_scalar(      # inv = rm*st*6 in-place
                        r_t[:, sl], r_t[:, sl], st_t[:], 6.0,
                        op0=mybir.AluOpType.mult, op1=mybir.AluOpType.mult))
                    dve.wait_ge(s_dve, a)
                    b_ = tag(dve.tensor_copy(f8_q[:], r_t[:, sl]))
                    dve.wait_ge(s_dve, b_)
                    c_ = tag(dve.tensor_copy(sb_q[:], f8_q[:]))   # s_b
                    dve.wait_ge(s_dve, c_)
                    d_ = tag(dve.tensor_scalar(     # c = s_b/s_t -> m_t
                        m_t[:, sl], sb_q[:], rt_t[:], None,
                        op0=mybir.AluOpType.mult))
                    e_ = tag(dve.reciprocal(t1_q[:], sb_q[:]))    # 1/s_b
                    dve.wait_ge(s_dve, e_)
                    f_ = tag(dve.tensor_scalar(     # nic16 = (1/s_b)*(-s_t)
                        nic16[:, sl], t1_q[:], nst_t[:], None,
                        op0=mybir.AluOpType.mult))
                    dve.wait_ge(s_dve, f_)
                    dve.wait_ge(s_dve, d_)

                    for tp in range(q * TPQ, (q + 1) * TPQ, 2):
                        pair = (tp, tp + 1)
                        ty = [0, 0]
                        tb_ = [0, 0]
                        for i, t in enumerate(pair):
                            if t == 0:
                                dve.wait_ge(s_act, TA)   # x16 fully written
                            u = t - 2                    # yb/pb slot reuse
                            if u >= 0:
                                if e_dve(u):
                                    pass                 # same-engine order
                                else:
                                    dve.wait_ge(s_pool, ptag6[u])
                            ty[i] = tag(dve.tensor_tensor(
                                b3(yb[t % 2][:]),
                                b3(x16[:, t * FB:(t + 1) * FB]),
                                bc(m_t[:, t * fbB:(t + 1) * fbB], fbB),
                                op=mybir.AluOpType.mult))
                        for i, t in enumerate(pair):
                            dve.wait_ge(s_dve, ty[i])
                            tb_[i] = tag(dve.tensor_scalar(
                                pb[t % 2][:], yb[t % 2][:].bitcast(u16),
                                M16, None,
                                op0=mybir.AluOpType.bitwise_and))
                        for i, t in enumerate(pair):
                            dve.wait_ge(s_dve, tb_[i])
                            tb_[i] = tag(dve.tensor_scalar(
                                pb[t % 2][:], pb[t % 2][:], A16, MN16,
                                op0=mybir.AluOpType.add,
                                op1=mybir.AluOpType.max))
                        for i, t in enumerate(pair):
                            dve.wait_ge(s_dve, tb_[i])
                            ty[i] = tag(dve.tensor_tensor(   # t -> yb slot
                                yb[t % 2][:], yb[t % 2][:],
                                pb[t % 2][:].bitcast(f16),
                                op=mybir.AluOpType.add))
                        for i, t in enumerate(pair):
                            dve.wait_ge(s_dve, ty[i])
                            tag_nq[t] = tag(dve.tensor_tensor(  # nq -> yb
                                yb[t % 2][:], pb[t % 2][:].bitcast(f16),
                                yb[t % 2][:],
                                op=mybir.AluOpType.subtract))
                        for i, t in enumerate(pair):
                            if not e_dve(t):
                                continue
                            dve.wait_ge(s_dve, tag_nq[t])
                            if t >= n_o:
                                dve.wait_ge(s_ob[t % n_o],
                                            16 * ((t - n_o) // n_o + 1))
                            tag6[t] = tag(dve.tensor_tensor(
                                b3(ob[t % n_o][:]),
                                b3(yb[t % 2][:]),
                                bc(nic16[:, t * fbB:(t + 1) * fbB], fbB),
                                op=mybir.AluOpType.mult))

            @block.gpsimd
            def _(pool):
                pcnt = 0
                ins = pool.memset(gall[0:1, :], 0.0)
                ins.then_inc(s_pool)
                pcnt += 1
                pool.wait_ge(s_pool, pcnt)
                pool.dma_start(out=cc_warm_in[:, :],
                               in_=gall[0:1, :]).then_inc(s_warm, 16)
                pool.wait_ge(s_warm, 16)
                pool.collective_compute(
                    "AllReduce",
                    mybir.AluOpType.max,
                    replica_groups=[list(range(n_cores))],
                    ins=[cc_warm_in.ap().opt()],
                    outs=[cc_warm_out.ap().opt()],
                ).then_inc(s_cc)
                pool.wait_ge(s_cdma, 16)        # cc_in staged
                pool.collective_compute(
                    "AllReduce",
                    mybir.AluOpType.max,
                    replica_groups=[list(range(n_cores))],
                    ins=[cc_in.ap().opt()],
                    outs=[cc_out.ap().opt()],
                ).then_inc(s_cc)

                for t in range(TB):
                    if e_dve(t):
                        continue
                    pool.wait_ge(s_dve, tag_nq[t])
                    if t >= n_o:
                        pool.wait_ge(s_ob[t % n_o],
                                     16 * ((t - n_o) // n_o + 1))
                    pool.tensor_tensor(
                        b3(ob[t % n_o][:]),
                        b3(yb[t % 2][:]),
                        bc(nic16[:, t * fbB:(t + 1) * fbB], fbB),
                        op=mybir.AluOpType.mult).then_inc(s_pool)
                    pcnt += 1
                    assert pcnt == ptag6[t]

            @block.scalar
            def _(act):
                Copy = mybir.ActivationFunctionType.Copy
                for t in range(TA):
                    if t >= n_xa:
                        act.wait_ge(s_dve, tagA[t - n_xa])
                    act.dma_start(
                        out=xa[t % n_xa][:, :],
                        in_=x_ext[:, t * FA:(t + 1) * FA],
                    ).then_inc(s_xa[t % n_xa], 16)
                    if t >= 1:
                        u = t - 1
                        act.wait_ge(s_xa[u % n_xa], 16 * (u // n_xa + 1))
                        act.activation(
                            x16[:, u * FA:(u + 1) * FA], xa[u % n_xa][:],
                            Copy).then_inc(s_act)
                u = TA - 1
                act.wait_ge(s_xa[u % n_xa], 16 * (u // n_xa + 1))
                act.activation(
                    x16[:, u * FA:(u + 1) * FA], xa[u % n_xa][:],
                    Copy).then_inc(s_act)

            @block.sync
            def _(sync):
                sync.wait_ge(s_dve, K_mx[0])
                sync.dma_start(out=cc_in[:, :], in_=mx_t[:, :]).then_inc(
                    s_cdma, 16)
                sync.wait_ge(s_cc, 2)
                sync.dma_start(
                    out=gall[:, :],
                    in_=cc_out.ap().broadcast_to([P, 128]),
                ).then_inc(s_cdma, 16)
                for t in range(TB):
                    if e_dve(t):
                        sync.wait_ge(s_dve, tag6[t])
                    else:
                        sync.wait_ge(s_pool, ptag6[t])
                    sync.dma_start(
                        out=out_ext[:, t * FB:(t + 1) * FB],
                        in_=ob[t % n_o][:, :],
                    ).then_inc(s_ob[t % n_o], 16)
                for i in range(n_o):
                    uses = len([t for t in range(TB) if t % n_o == i])
                    sync.wait_ge(s_ob[i], 16 * uses)

    return nc


_CACHE = {}


def _get_nc():
    if "nc" not in _CACHE:
        _CACHE["nc"] = build_nc()
    return _CACHE["nc"]


def kernel(x: np.ndarray) -> np.ndarray:
    from concourse.bass_utils import run_bass_kernel_spmd

    x = np.asarray(x, dtype=np.float32)
    assert x.shape == FULL_SHAPE
    shards = x.reshape(N_CORES, P, L)
    in_maps = [{"x": np.ascontiguousarray(shards[i])} for i in range(N_CORES)]
    nc = _get_nc()
    res = run_bass_kernel_spmd(nc, in_maps, core_ids=list(range(N_CORES)))
    out = np.stack([r["out"] for r in res.results], axis=0)
    return out.reshape(FULL_SHAPE)


# revision 8
# speedup vs baseline: 1.1164x; 1.1164x over previous
"""NVFP4-style activation quantizer on 8 TRN2 NeuronCores (raw bass).

Single-pass over HBM: x streams in once; an fp16 copy (x16) of the whole
shard stays resident in SBUF, so the post-AllReduce quantize pass re-reads
nothing from HBM (67MB traffic/core instead of 100MB for two-pass).

Reference semantics (per 16-element block, fp32):
    s_t  = max|x| / (6*448)                      (global, needs all-reduce)
    m_b  = max|x| over block
    s_b  = fp8_e4m3_roundtrip(6 * s_t / m_b)
    out  = sign(x) * fp4_121(|x|/s_t * s_b) / s_b * s_t

Device algorithm per element (signed, select-free), fp16 magic rounding:
    y16 = fp16(x16 * c)        c = s_b / s_t  (per block, fp32)
    p   = (bits(y16) & 0x7C00) + 0x2600        (u16 ops)
    B   = max(p, 0x6200)                       (= 768*max(2^e,1) as fp16)
    t   = fp16(y16 + B)        (fp16 RNE add rounds y to the fp4 grid)
    nq  = B - t                (exact; = -fp4_121(y)*sign)
    out = nq * nic             nic = -s_t/s_b  (per block, fp16)

Measured rel_err vs the fp32 reference on seed-0 data: 8.2e-3.

Engine split (from measured per-op costs: DVE TT 1.27us/[P,1024] at 1x,
DVE TS 0.60us at 2x, Pool TT 2.45us, no DVE fast mode for TT):
  SYNC   input DMAs + collective staging + output DMAs
  ACT    fp16 conversion of x into resident x16 (nothing else)
  DVE    pass-A block maxes, per-block scales, the u16/fp16 rounding ops
         (and/add+max/t/nq) for all tiles, plus a small share of the two
         per-block broadcast multiplies
  POOL   AllReduce (warmed up) + most broadcast multiplies (y16 and the
         final nq*nic), balancing DVE
"""

import numpy as np

FULL_SHAPE = (4, 4096, 4096)
N_CORES = 8
P = 128
TOTAL = 4 * 4096 * 4096
L = TOTAL // (N_CORES * P)   # 65536 elements per partition per core
NBLK = L // 16               # 4096 blocks per partition

M16 = 0x7C00                 # fp16 exponent mask
A16 = 0x2600                 # exponent +9, set mantissa bit -> *768
MN16 = 0x6200                # bits of fp16 768.0


def build_nc(n_cores=N_CORES, FA=2048, FB=1024, n_xa=2, n_o=2,
             n_chunks=8, yp_mod=27, sp_mod=27):
    """yp_mod/sp_mod: of every 32 tiles, how many get their y16 / final
    multiply executed on Pool (the rest stay on DVE)."""
    from contextlib import ExitStack

    import concourse.bass as bass
    from concourse import mybir

    f32 = mybir.dt.float32
    f16 = mybir.dt.float16
    u16 = mybir.dt.uint16
    f8 = mybir.dt.float8e4

    TA = L // FA                 # pass-A tiles (32)
    TB = L // FB                 # pass-B tiles (64)
    fbB = FB // 16               # blocks per pass-B tile (64)
    QN = NBLK // n_chunks        # blocks per scale chunk (512)
    TPQ = TB // n_chunks         # pass-B tiles per chunk (8)
    assert L % FA == 0 and L % FB == 0 and NBLK % n_chunks == 0
    assert TB % n_chunks == 0 and TPQ % 2 == 0

    def y_pool(t):
        return (t % 32) < yp_mod

    def s6_pool(t):
        return (t % 32) < sp_mod

    # pool cumulative tag schedule (DVE traced first, needs these).
    # pool stream: warmup memset (+1), then per tile t: [y16(t) if pool],
    # [step6(t-1) if pool]; trailing step6(TB-1).
    ptagy = [0] * TB
    ptag6 = [0] * TB
    pc = 1
    for t in range(TB):
        if y_pool(t):
            pc += 1
        ptagy[t] = pc
        if t >= 1 and s6_pool(t - 1):
            pc += 1
            ptag6[t - 1] = pc
    if s6_pool(TB - 1):
        pc += 1
        ptag6[TB - 1] = pc

    nc = bass.Bass(num_devices=n_cores, debug=False)
    x_ext = nc.declare_dram_parameter("x", [P, L], f32, isOutput=False)
    out_ext = nc.declare_dram_parameter("out", [P, L], f32, isOutput=True)
    cc_in = nc.dram_tensor("cc_in", [1, 128], f32)
    cc_out = nc.dram_tensor("cc_out", [1, 128], f32, addr_space="Shared")
    cc_warm_in = nc.dram_tensor("cc_warm_in", [1, 128], f32)
    cc_warm_out = nc.dram_tensor("cc_warm_out", [1, 128], f32,
                                 addr_space="Shared")

    with ExitStack() as ctx:
        def sem(name):
            return ctx.enter_context(nc.semaphore(name))

        def sbuf(name, shape, dt=f32):
            return ctx.enter_context(nc.sbuf_tensor(name, shape, dt))

        s_xa = [sem(f"s_xa{i}") for i in range(n_xa)]   # input DMAs   (+16)
        s_ob = [sem(f"s_ob{i}") for i in range(n_o)]    # output DMAs  (+16)
        s_cdma = sem("s_cdma")   # collective staging DMAs             (+16)
        s_act = sem("s_act")     # ACT x16 conversions                 (+1)
        s_dve = sem("s_dve")     # tagged DVE ops                      (+1)
        s_pool = sem("s_pool")   # pool ops                            (+1)
        s_cc = sem("s_cc")       # collectives                         (+1)
        s_warm = sem("s_warm")   # warm-up staging dma                 (+16)

        xa = [sbuf(f"xa{i}", [P, FA]) for i in range(n_xa)]
        x16 = sbuf("x16", [P, L], f16)        # resident fp16 shard (128KB)
        m_t = sbuf("m_t", [P, NBLK])          # block max m -> c
        r_t = sbuf("r_t", [P, NBLK])          # rm=1/m (AR window) -> inv
        nic16 = sbuf("nic16", [P, NBLK], f16)
        f8_q = sbuf("f8_q", [P, QN], f8)      # chunk temps
        sb_q = sbuf("sb_q", [P, QN])
        t1_q = sbuf("t1_q", [P, QN])
        yb = [sbuf(f"yb{i}", [P, FB], f16) for i in range(2)]
        pb = [sbuf(f"pb{i}", [P, FB], u16) for i in range(2)]
        ob = [sbuf(f"ob{i}", [P, FB]) for i in range(n_o)]
        gall = sbuf("gall", [P, 128])
        mx_t = sbuf("mx_t", [P, 1])
        g128 = sbuf("g128", [P, 1])
        st_t = sbuf("st_t", [P, 1])
        rt_t = sbuf("rt_t", [P, 1])
        nst_t = sbuf("nst_t", [P, 1])

        tagA = [0] * TA
        tag_y = [0] * TB         # DVE y16 tags (when on DVE)
        tag_nq = [0] * TB        # nq (last rounding op, yb slot)
        tag6 = [0] * TB          # DVE final-multiply tags (when on DVE)
        K_mx = [0]
        qc_tag = [0] * n_chunks  # c chunk ready (DVE tag)
        qn_tag = [0] * n_chunks  # nic16 chunk ready (DVE tag)

        def b3(ap):
            return ap.rearrange("p (b s) -> p b s", s=16)

        def bc(ap_slice, nb):
            return ap_slice.unsqueeze(-1).broadcast_to([P, nb, 16])

        def y_args(t):
            return dict(
                out=b3(yb[t % 2][:]),
                in0=b3(x16[:, t * FB:(t + 1) * FB]),
                in1=bc(m_t[:, t * fbB:(t + 1) * fbB], fbB),
            )

        def s6_args(t):
            return dict(
                out=b3(ob[t % n_o][:]),
                in0=b3(yb[t % 2][:]),
                in1=bc(nic16[:, t * fbB:(t + 1) * fbB], fbB),
            )

        with nc.Block() as block:

            @block.vector
            def _(dve):
                cnt = 0

                def tag(ins):
                    nonlocal cnt
                    ins.then_inc(s_dve)
                    cnt += 1
                    return cnt

                # ---- pass A: per-block abs max ----
                fbA = FA // 16
                for t in range(TA):
                    dve.wait_ge(s_xa[t % n_xa], 16 * (t // n_xa + 1))
                    tagA[t] = tag(dve.tensor_reduce(
                        out=m_t[:, t * fbA:(t + 1) * fbA],
                        in_=b3(xa[t % n_xa][:]),
                        axis=mybir.AxisListType.X,
                        op=mybir.AluOpType.max,
                        apply_absolute_value=True,
                    ))
                dve.wait_ge(s_dve, tagA[TA - 1])
                K_mx[0] = tag(dve.tensor_reduce(
                    out=mx_t[:], in_=m_t[:], axis=mybir.AxisListType.X,
                    op=mybir.AluOpType.max))
                dve.wait_ge(s_dve, K_mx[0])
                # rm = 1/m for the whole shard, hidden in the AllReduce window
                h = NBLK // 2
                k_rm0 = tag(dve.reciprocal(r_t[:, 0:h], m_t[:, 0:h]))
                k_rm1 = tag(dve.reciprocal(r_t[:, h:NBLK], m_t[:, h:NBLK]))

                # ---- post-AllReduce scalars ----
                dve.wait_ge(s_cdma, 32)        # gall loaded (bcast DMA)
                k1 = tag(dve.tensor_reduce(
                    out=g128[:], in_=gall[:], axis=mybir.AxisListType.X,
                    op=mybir.AluOpType.max))
                dve.wait_ge(s_dve, k1)
                k2 = tag(dve.tensor_scalar(
                    st_t[:], g128[:], 1.0 / 2688.0, None,
                    op0=mybir.AluOpType.mult))
                dve.wait_ge(s_dve, k2)
                k3 = tag(dve.reciprocal(rt_t[:], st_t[:]))
                k4 = tag(dve.tensor_scalar(
                    nst_t[:], st_t[:], -1.0, None, op0=mybir.AluOpType.mult))
                dve.wait_ge(s_dve, k4)
                dve.wait_ge(s_dve, k3)
                dve.wait_ge(s_dve, k_rm0)
                dve.wait_ge(s_dve, k_rm1)
                seen_act = [False]

                # ---- per-chunk scales + pass-B tiles ----
                for q in range(n_chunks):
                    sl = slice(q * QN, (q + 1) * QN)
                    a = tag(dve.tensor_scalar(      # inv = rm*st*6 in-place
                        r_t[:, sl], r_t[:, sl], st_t[:], 6.0,
                        op0=mybir.AluOpType.mult, op1=mybir.AluOpType.mult))
                    dve.wait_ge(s_dve, a)
                    b_ = tag(dve.tensor_copy(f8_q[:], r_t[:, sl]))
                    dve.wait_ge(s_dve, b_)
                    c_ = tag(dve.tensor_copy(sb_q[:], f8_q[:]))   # s_b
                    dve.wait_ge(s_dve, c_)
                    d_ = tag(dve.tensor_scalar(     # c = s_b/s_t -> m_t
                        m_t[:, sl], sb_q[:], rt_t[:], None,
                        op0=mybir.AluOpType.mult))
                    e_ = tag(dve.reciprocal(t1_q[:], sb_q[:]))    # 1/s_b
                    dve.wait_ge(s_dve, e_)
                    f_ = tag(dve.tensor_scalar(     # nic16 = (1/s_b)*(-s_t)
                        nic16[:, sl], t1_q[:], nst_t[:], None,
                        op0=mybir.AluOpType.mult))
                    dve.wait_ge(s_dve, f_)
                    dve.wait_ge(s_dve, d_)
                    qc_tag[q] = d_
                    qn_tag[q] = f_

                    for tp in range(q * TPQ, (q + 1) * TPQ, 2):
                        pair = (tp, tp + 1)
                        tw = [0, 0]
                        for i, t in enumerate(pair):
                            if y_pool(t):
                                continue
                            if not seen_act[0]:
                                seen_act[0] = True
                                dve.wait_ge(s_act, TA)   # x16 fully written
                            u = t - 2                    # yb slot reuse
                            if u >= 0:
                                if s6_pool(u):
                                    dve.wait_ge(s_pool, ptag6[u])
                            tag_y[t] = tag(dve.tensor_tensor(
                                op=mybir.AluOpType.mult, **y_args(t)))
                        for i, t in enumerate(pair):
                            if y_pool(t):
                                dve.wait_ge(s_pool, ptagy[t])
                            else:
                                dve.wait_ge(s_dve, tag_y[t])
                            tw[i] = tag(dve.tensor_scalar(
                                pb[t % 2][:], yb[t % 2][:].bitcast(u16),
                                M16, None,
                                op0=mybir.AluOpType.bitwise_and))
                        for i, t in enumerate(pair):
                            dve.wait_ge(s_dve, tw[i])
                            tw[i] = tag(dve.tensor_scalar(
                                pb[t % 2][:], pb[t % 2][:], A16, MN16,
                                op0=mybir.AluOpType.add,
                                op1=mybir.AluOpType.max))
                        for i, t in enumerate(pair):
                            dve.wait_ge(s_dve, tw[i])
                            tw[i] = tag(dve.tensor_tensor(   # t -> yb slot
                                yb[t % 2][:], yb[t % 2][:],
                                pb[t % 2][:].bitcast(f16),
                                op=mybir.AluOpType.add))
                        for i, t in enumerate(pair):
                            dve.wait_ge(s_dve, tw[i])
                            tag_nq[t] = tag(dve.tensor_tensor(  # nq -> yb
                                yb[t % 2][:], pb[t % 2][:].bitcast(f16),
                                yb[t % 2][:],
                                op=mybir.AluOpType.subtract))
                        for i, t in enumerate(pair):
                            if s6_pool(t):
                                continue
                            dve.wait_ge(s_dve, tag_nq[t])
                            if t >= n_o:
                                dve.wait_ge(s_ob[t % n_o],
                                            16 * ((t - n_o) // n_o + 1))
                            tag6[t] = tag(dve.tensor_tensor(
                                op=mybir.AluOpType.mult, **s6_args(t)))

            @block.gpsimd
            def _(pool):
                pcnt = 0
                ins = pool.memset(gall[0:1, :], 0.0)
                ins.then_inc(s_pool)
                pcnt += 1
                pool.wait_ge(s_pool, pcnt)
                pool.dma_start(out=cc_warm_in[:, :],
                               in_=gall[0:1, :]).then_inc(s_warm, 16)
                pool.wait_ge(s_warm, 16)
                pool.collective_compute(
                    "AllReduce",
                    mybir.AluOpType.max,
                    replica_groups=[list(range(n_cores))],
                    ins=[cc_warm_in.ap().opt()],
                    outs=[cc_warm_out.ap().opt()],
                ).then_inc(s_cc)
                pool.wait_ge(s_cdma, 16)        # cc_in staged
                pool.collective_compute(
                    "AllReduce",
                    mybir.AluOpType.max,
                    replica_groups=[list(range(n_cores))],
                    ins=[cc_in.ap().opt()],
                    outs=[cc_out.ap().opt()],
                ).then_inc(s_cc)

                def pstep6(t):
                    nonlocal pcnt
                    if (t % TPQ) == 0:
                        pool.wait_ge(s_dve, qn_tag[t // TPQ])
                    pool.wait_ge(s_dve, tag_nq[t])
                    if t >= n_o:
                        pool.wait_ge(s_ob[t % n_o],
                                     16 * ((t - n_o) // n_o + 1))
                    pool.tensor_tensor(
                        op=mybir.AluOpType.mult, **s6_args(t)).then_inc(
                        s_pool)
                    pcnt += 1
                    assert pcnt == ptag6[t]

                for t in range(TB):
                    if y_pool(t):
                        if t == 0:
                            pool.wait_ge(s_act, TA)
                        if (t % TPQ) == 0:
                            pool.wait_ge(s_dve, qc_tag[t // TPQ])
                        u = t - 2
                        if u >= 0 and not s6_pool(u):
                            pool.wait_ge(s_dve, tag6[u])
                        pool.tensor_tensor(
                            op=mybir.AluOpType.mult, **y_args(t)).then_inc(
                            s_pool)
                        pcnt += 1
                        assert pcnt == ptagy[t]
                    if t >= 1 and s6_pool(t - 1):
                        pstep6(t - 1)
                if s6_pool(TB - 1):
                    pstep6(TB - 1)

            @block.scalar
            def _(act):
                Copy = mybir.ActivationFunctionType.Copy
                for t in range(TA):
                    act.wait_ge(s_xa[t % n_xa], 16 * (t // n_xa + 1))
                    act.activation(
                        x16[:, t * FA:(t + 1) * FA], xa[t % n_xa][:],
                        Copy).then_inc(s_act)

            @block.sync
            def _(sync):
                # input DMAs: slot free when DVE's reduce AND ACT's copy of
                # the previous occupant are both done
                for t in range(TA):
                    if t >= n_xa:
                        sync.wait_ge(s_dve, tagA[t - n_xa])
                        sync.wait_ge(s_act, t - n_xa + 1)
                    sync.dma_start(
                        out=xa[t % n_xa][:, :],
                        in_=x_ext[:, t * FA:(t + 1) * FA],
                    ).then_inc(s_xa[t % n_xa], 16)
                sync.wait_ge(s_dve, K_mx[0])
                sync.dma_start(out=cc_in[:, :], in_=mx_t[:, :]).then_inc(
                    s_cdma, 16)
                sync.wait_ge(s_cc, 2)
                sync.dma_start(
                    out=gall[:, :],
                    in_=cc_out.ap().broadcast_to([P, 128]),
                ).then_inc(s_cdma, 16)
                for t in range(TB):
                    if s6_pool(t):
                        sync.wait_ge(s_pool, ptag6[t])
                    else:
                        sync.wait_ge(s_dve, tag6[t])
                    sync.dma_start(
                        out=out_ext[:, t * FB:(t + 1) * FB],
                        in_=ob[t % n_o][:, :],
                    ).then_inc(s_ob[t % n_o], 16)
                for i in range(n_o):
                    uses = len([t for t in range(TB) if t % n_o == i])
                    sync.wait_ge(s_ob[i], 16 * uses)

    return nc


_CACHE = {}


def _get_nc():
    if "nc" not in _CACHE:
        _CACHE["nc"] = build_nc()
    return _CACHE["nc"]


def kernel(x: np.ndarray) -> np.ndarray:
    from concourse.bass_utils import run_bass_kernel_spmd

    x = np.asarray(x, dtype=np.float32)
    assert x.shape == FULL_SHAPE
    shards = x.reshape(N_CORES, P, L)
    in_maps = [{"x": np.ascontiguousarray(shards[i])} for i in range(N_CORES)]
    nc = _get_nc()
    res = run_bass_kernel_spmd(nc, in_maps, core_ids=list(range(N_CORES)))
    out = np.stack([r["out"] for r in res.results], axis=0)
    return out.reshape(FULL_SHAPE)
